# revision 13
# baseline (speedup 1.0000x reference)
"""Trainium2 Bass kernel for nn_Chord_Note_Conv — v2 (restructured conv1).

v1 folded both embedding tables into conv1, paying a 1024-bin x 3-tap x
256-out contraction per position (48 matmuls / 512-pos s-block).  v2 cuts PE
cycles ~25% by splitting the work:
  * note path: 896-bin one-hot matmul builds ne[256] once (14 mm/s-block),
    then a dense 256-ch 3-tap conv1 (12 mm/s-block);
  * chord path: a 450-bin "concat 3 taps" one-hot (prev/cur/next chord per
    position, host-shifted pre-offset indices) folded with E_c@W1_k,
    contracted column-exact (8 mm/s-block);
  * conv2 + fc unchanged.  Per-s-block matmuls: 58 -> 44.
One GPSIMD local_scatter per 128-pos tile builds all 1408 one-hot bins
(16 deduped note slots + 3 chord slots); HWDGE xbar transpose moves them to
bin-major.  Index prep runs entirely in fp16 on DVE via a 16x16 outer-compare
(few big ops — DVE is dispatch-bound on small ones).  All inter-stage buffers
are persistent SBUF (tile pools tied producer engines to PE progress), the
pipeline is skewed so every stage's inputs are a full iteration old, and halo
guard copies ride the Scalar queue that produces them.
"""

import os

os.environ.setdefault("MYCRO_LOCAL_CACHE", "1")

import numpy as np

try:
    import concourse.bass as bass  # noqa: F401
except ImportError:
    import sys

    sys.path.insert(0, "/opt/trn_rl_repo")
    import concourse.bass as bass

from concourse import bacc, mybir, tile
from concourse.bass_utils import run_bass_kernel_spmd

FP16 = mybir.dt.float16   # NB: bf16 produced wrong results on HW (and was no faster)
F32 = mybir.dt.float32
I16 = mybir.dt.int16
ALU = mybir.AluOpType

CHORD_SIZE, NOTE_SIZE = 150, 832
B, S, N = 16, 2048, 16
NCORES = 8
BLOC = B // NCORES          # batch rows per core
P = BLOC * S                # positions per core = 4096
NT = P // 128               # pos tiles per core = 32
NSB = P // 512              # s-blocks per core = 8
SB_PER_BATCH = S // 512     # 4
NBN = 896                   # note bins (832 + pad to 7*128)
NQN = NBN // 128            # 7 note chunks
NBC = 512                   # chord-concat bins (3*150 + pad to 4*128)
NQC = NBC // 128            # 4 chord chunks
NB = NBN + NBC              # 1408 total bins = 11 chunks
NQ = NB // 128              # 11
CW = 544                    # ne/x2 s-block buffer width (16 guard + 512 + pad)


def _build_program():
    nc = bacc.Bacc("TRN2", target_bir_lowering=False, debug=False,
                   enable_asserts=False, num_devices=NCORES)

    # ---- DRAM I/O (flat 2D) ----
    d_note = nc.dram_tensor("note16", [128, NT * 16], FP16, kind="ExternalInput")
    d_csidx = nc.dram_tensor("csidx", [128, 3 * NT], I16, kind="ExternalInput")
    d_fne = nc.dram_tensor("fne", [128, NQN * 2 * 128], FP16, kind="ExternalInput")
    d_w1ne = nc.dram_tensor("w1ne", [128, 2 * 3 * 2 * 128], FP16, kind="ExternalInput")
    d_fce = nc.dram_tensor("fce", [128, NQC * 2 * 128], FP16, kind="ExternalInput")
    d_w2t = nc.dram_tensor("w2t", [128, 3 * 2 * 64], FP16, kind="ExternalInput")
    d_fcwb = nc.dram_tensor("fcwb", [65, 152], FP16, kind="ExternalInput")
    d_b1 = nc.dram_tensor("b1t", [128, 2], F32, kind="ExternalInput")
    d_b2 = nc.dram_tensor("b2t", [64, 1], F32, kind="ExternalInput")
    d_prepc = nc.dram_tensor("prepc", [128, 528], FP16, kind="ExternalInput")
    d_ones = nc.dram_tensor("onesr", [1, P], FP16, kind="ExternalInput")
    d_out = nc.dram_tensor("out", [P, CHORD_SIZE], F32, kind="ExternalOutput")

    # ---- persistent SBUF ----
    def sb(name, shape, dt):
        return nc.alloc_sbuf_tensor(name, list(shape), dt).ap()

    s_note = sb("s_note", [128, NT * 16], FP16)
    s_csidx = sb("s_csidx", [128, 3 * NT], I16)
    s_fne = sb("s_fne", [128, NQN * 2 * 128], FP16)
    s_w1ne = sb("s_w1ne", [128, 2 * 3 * 2 * 128], FP16)
    s_fce = sb("s_fce", [128, NQC * 2 * 128], FP16)
    s_w2t = sb("s_w2t", [128, 3 * 2 * 64], FP16)
    s_fcwb = sb("s_fcwb", [65, 152], FP16)
    s_b1 = sb("s_b1", [128, 2], F32)
    s_b2 = sb("s_b2", [64, 1], F32)
    s_prepc = sb("s_prepc", [128, 528], FP16)
    s_eqall = sb("s_eqall", [128, 4 * 256], FP16)
    s_eqall2 = sb("s_eqall2", [128, 4 * 256], FP16)
    s_x3 = sb("s_x3", [65, P], FP16)
    s_mask = sb("s_mask", [128, NT * 16], FP16)
    s_val = sb("s_val", [128, NT * 16], FP16)
    s_isdup = sb("s_isdup", [128, NT * 16], FP16)
    s_eq = sb("s_eq", [128, NT * 16], FP16)
    s_tmp = sb("s_tmp", [128, NT * 16], FP16)
    s_cnt = sb("s_cnt", [128, NT], F32)
    s_inv = sb("s_inv", [128, NT], F32)
    s_inv16 = sb("s_inv16", [128, NT], FP16)
    s_sidx = sb("s_sidx", [128, NT * 20], I16)
    s_sval = sb("s_sval", [128, NT * 20], FP16)
    # per-s-block persistent: ne [128, 2q, CW], x2 [128, 2q, CW]
    s_ne = [sb(f"s_ne{i}", [128, 2 * CW], FP16) for i in range(NSB)]
    s_x2 = [sb(f"s_x2{i}", [128, 2 * CW], FP16) for i in range(NSB)]
    # persistent hist-path buffers (pools here would tie scatter/transpose
    # recycling to PE progress via conservative pool-WAR sync)
    s_hb = [sb(f"s_hb{i}", [128, NB], FP16) for i in range(8)]
    s_ct = [sb(f"s_ct{i}", [128, NQ * 512], FP16) for i in range(NSB)]

    note3 = s_note.rearrange("p (c j) -> p c j", j=16)
    mask3 = s_mask.rearrange("p (c j) -> p c j", j=16)
    val3 = s_val.rearrange("p (c j) -> p c j", j=16)
    isd3 = s_isdup.rearrange("p (c j) -> p c j", j=16)
    eq3 = s_eq.rearrange("p (c j) -> p c j", j=16)
    tmp3 = s_tmp.rearrange("p (c j) -> p c j", j=16)
    sidx3 = s_sidx.rearrange("p (c j) -> p c j", j=20)
    sval3 = s_sval.rearrange("p (c j) -> p c j", j=20)
    csidx3 = s_csidx.rearrange("p (c k) -> p c k", k=3)
    fne4 = s_fne.rearrange("p (q o m) -> p q o m", q=NQN, o=2)
    w1ne5 = s_w1ne.rearrange("p (q k o m) -> p q k o m", q=2, k=3, o=2)
    fce4 = s_fce.rearrange("p (q o m) -> p q o m", q=NQC, o=2)
    w2t4 = s_w2t.rearrange("p (k q o) -> p k q o", k=3, q=2)
    ne3 = [t.rearrange("p (q w) -> p q w", w=CW) for t in s_ne]
    x23 = [t.rearrange("p (q w) -> p q w", w=CW) for t in s_x2]
    ct3 = [t.rearrange("p (q w) -> p q w", w=512) for t in s_ct]

    with tile.TileContext(nc) as tc, \
         nc.allow_low_precision(reason="int16 counts <=16 are exact; fp16 data"):
        v = nc.vector
        # ---- input loads (latency-critical first) ----
        nc.sync.dma_start(s_note, d_note.ap())
        nc.sync.dma_start(s_csidx, d_csidx.ap())
        nc.sync.dma_start(s_prepc, d_prepc.ap())
        nc.sync.dma_start(s_fne, d_fne.ap())
        nc.sync.dma_start(s_fce, d_fce.ap())
        nc.sync.dma_start(s_w1ne, d_w1ne.ap())
        nc.sync.dma_start(s_w2t, d_w2t.ap())
        nc.sync.dma_start(s_fcwb, d_fcwb.ap())
        nc.sync.dma_start(s_b1, d_b1.ap())
        nc.sync.dma_start(s_b2, d_b2.ap())
        nc.sync.dma_start(s_x3[64:65, :], d_ones.ap())  # fc bias ones row

        # chord scatter slots: idx from host (pre-offset, -1 at boundaries),
        # val 1.0; slot 19 is inert padding
        v.tensor_copy(sidx3[:, :, 16:19], csidx3)
        v.memset(sval3[:, :, 16:19], 1.0)
        v.memset(sidx3[:, :, 19:20], -1)
        v.memset(sval3[:, :, 19:20], 0.0)

        def static_halo_zeros():
            # batch-boundary halo zeros (never rewritten)
            for k in range(NSB):
                if k % SB_PER_BATCH == 0:
                    v.memset(ne3[k][:, :, 15:16], 0.0)
                    v.memset(x23[k][:, :, 15:16], 0.0)
                if k % SB_PER_BATCH == SB_PER_BATCH - 1:
                    v.memset(ne3[k][:, :, 528:529], 0.0)
                    v.memset(x23[k][:, :, 528:529], 0.0)

        # ---- index prep ----
        def finish_prep(a, b):
            """common tail: fv, scat_idx, scat_val from mask/val/isdup."""
            w_ = b - a
            nt = note3[:, a:b, :]
            mk = mask3[:, a:b, :]
            isd = isd3[:, a:b, :]
            eq = eq3[:, a:b, :]
            tp = tmp3[:, a:b, :]
            v.tensor_tensor(tp, mk, isd, ALU.mult)
            v.tensor_tensor(isd, mk, tp, ALU.subtract)   # fv
            v.tensor_tensor(tp, nt, isd, ALU.mult)
            v.tensor_scalar_add(eq, isd, -1)
            v.tensor_tensor(sidx3[:, a:b, 0:16], tp, eq, ALU.add)
            inv_b = s_inv16[:, a:b].unsqueeze(2).broadcast_to((128, w_, 16))
            return v.tensor_tensor(sval3[:, a:b, 0:16], val3[:, a:b, :], inv_b,
                                   ALU.mult)

        ut2 = s_prepc[:, 0:256].rearrange("p (a b) -> p a b", b=16)    # 1[j' >= j]
        lt2 = s_prepc[:, 256:512].rearrange("p (a b) -> p a b", b=16)  # 1[j' < j]
        jidx = s_prepc[:, 512:528]                                     # iota 0..15

        def head_prep(a, b):
            """Low-latency variant: 16x16 outer compare in few big ops."""
            w_ = b - a
            nt = note3[:, a:b, :]
            mk = mask3[:, a:b, :]
            eq = eq3[:, a:b, :]
            tp = tmp3[:, a:b, :]
            eqa = s_eqall.rearrange("p (c i j) -> p c i j", i=16, j=16)[:, 0:w_, :, :]
            eqb = s_eqall2.rearrange("p (c i j) -> p c i j", i=16, j=16)[:, 0:w_, :, :]
            jid_b = jidx.unsqueeze(1).broadcast_to((128, w_, 16))
            # first-zero index -> mask, cnt
            v.tensor_scalar(eq, nt, 0, None, ALU.not_equal)            # nz
            v.scalar_tensor_tensor(tp, eq, 16, jid_b, ALU.mult, ALU.add)
            v.tensor_reduce(s_cnt[:, a:b], tp, mybir.AxisListType.X, ALU.min)
            v.tensor_tensor(mk, jid_b,
                            s_cnt[:, a:b].unsqueeze(2).broadcast_to((128, w_, 16)),
                            ALU.is_lt)
            v.tensor_scalar(s_cnt[:, a:b], s_cnt[:, a:b], 16.0, 1.0, ALU.min, ALU.max)
            v.reciprocal(s_inv[:, a:b], s_cnt[:, a:b])
            v.tensor_copy(s_inv16[:, a:b], s_inv[:, a:b])
            # 16x16 equality outer product
            v.tensor_tensor(eqa,
                            nt.unsqueeze(3).broadcast_to((128, w_, 16, 16)),
                            nt.unsqueeze(2).broadcast_to((128, w_, 16, 16)),
                            ALU.is_equal)
            # t = m_j' * eq ; val_j = sum_{j'>=j} t ; isdup_j = max_{j'<j} t
            v.tensor_tensor(eqa, eqa,
                            mask3[:, a:b, :].unsqueeze(2).broadcast_to((128, w_, 16, 16)),
                            ALU.mult)
            v.tensor_tensor(eqb, eqa,
                            lt2.unsqueeze(1).broadcast_to((128, w_, 16, 16)), ALU.mult)
            v.tensor_reduce(isd3[:, a:b, :], eqb, mybir.AxisListType.X, ALU.max)
            v.tensor_tensor(eqa, eqa,
                            ut2.unsqueeze(1).broadcast_to((128, w_, 16, 16)), ALU.mult)
            v.tensor_reduce(val3[:, a:b, :], eqa, mybir.AxisListType.X, ALU.add)
            return finish_prep(a, b)

        # ---- histogram build (scatter notes+chord) + transpose
        # NB: all transposes must stay on the sync queue — issuing some
        # from the Scalar engine produced silent data races twice.
        # The chord-bin transposes are deferred ~2 s-blocks: the PE needs
        # note chunks (ne_build) ~20us before chord chunks (conv1), so
        # note-only transposes (896 cols, ~1.0us) feed the fill phase faster
        # than full-width ones (1408 cols, ~1.55us).
        def hist_tile(t):
            """One 128-pos tile: build [128, 1408] one-hot rows, transpose into
            the s-block ct buffer at cols 128*(t%4).  First two s-blocks:
            note bins only (chord deferred) — shorter ops feed ne_build(0/1)
            sooner; the ~700ns fixed cost per extra transpose op makes the
            split a loss in steady state, so later tiles go full-width."""
            hb = s_hb[t % 8]
            nc.gpsimd.local_scatter(
                hb, sval3[:, t, :], sidx3[:, t, :],
                channels=128, num_elems=NB, num_idxs=20)
            tq = t % 4
            if t < 8:
                nc.sync.dma_start_transpose(
                    ct3[t // 4][:, 0:NQN, 128 * tq: 128 * (tq + 1)],
                    hb[:, 0:NBN])
            else:
                nc.sync.dma_start_transpose(
                    ct3[t // 4][:, :, 128 * tq: 128 * (tq + 1)], hb)

        def hist_tile_chord(t):
            tq = t % 4
            nc.sync.dma_start_transpose(
                ct3[t // 4][:, NQN:NQ, 128 * tq: 128 * (tq + 1)],
                s_hb[t % 8][:, NBN:NB])

        # ---- conv/fc stages ----
        def sblock_stages(ct_of):
            def ne_build(sbk):
                ct = ct_of(sbk)
                # fill-phase s-blocks: column-halves so each matmul group only
                # needs two transposes (starts/feeds the PE earlier)
                cols = ((0, 256), (256, 512)) if sbk <= 1 else ((0, 512),)
                for oc in range(2):
                    for a, b in cols:
                        ps = ppn.tile([128, b - a], F32, tag="pn")
                        for q in range(NQN):
                            nc.tensor.matmul(ps[:], fne4[:, q, oc, :],
                                             ct[:, q, a:b],
                                             start=(q == 0),
                                             stop=(q == NQN - 1))
                        nc.scalar.copy(ne3[sbk][:, oc, 16 + a: 16 + b], ps[:])
                # halo guards on Scalar (all deps already satisfied here)
                if sbk % SB_PER_BATCH != 0:
                    nc.scalar.copy(ne3[sbk][:, :, 15:16],
                                   ne3[sbk - 1][:, :, 527:528])
                    nc.scalar.copy(ne3[sbk - 1][:, :, 528:529],
                                   ne3[sbk][:, :, 16:17])

            def conv1_sb(sbk):
                ct = ct_of(sbk)
                for co in range(2):
                    ps = pp1.tile([128, 512], F32, tag="p1")
                    mms = [("c", q, 0) for q in range(NQC)] + \
                          [("n", q, k) for q in range(2) for k in range(3)]
                    for i, (kind, q, k) in enumerate(mms):
                        if kind == "c":
                            lhs = fce4[:, q, co, :]
                            rhs = ct[:, NQN + q, 0:512]
                        else:
                            lhs = w1ne5[:, q, k, co, :]
                            rhs = ne3[sbk][:, q, 15 + k: 527 + k]
                        nc.tensor.matmul(ps[:], lhs, rhs,
                                         start=(i == 0), stop=(i == len(mms) - 1))
                    nc.scalar.activation(
                        x23[sbk][:, co, 16:528], ps[:],
                        mybir.ActivationFunctionType.Relu,
                        bias=s_b1[:, co:co + 1])
                if sbk % SB_PER_BATCH != 0:
                    nc.scalar.copy(x23[sbk][:, :, 15:16],
                                   x23[sbk - 1][:, :, 527:528])
                    nc.scalar.copy(x23[sbk - 1][:, :, 528:529],
                                   x23[sbk][:, :, 16:17])

            def conv2_sb(sbk):
                ps2 = pp2.tile([64, 512], F32, tag="p2")
                mms = [(1, 0), (0, 0), (2, 0), (0, 1), (1, 1), (2, 1)]
                for i, (k, q) in enumerate(mms):
                    nc.tensor.matmul(
                        ps2[:], w2t4[:, k, q, :],
                        x23[sbk][:, q, 15 + k: 527 + k],
                        start=(i == 0), stop=(i == len(mms) - 1))
                nc.scalar.activation(
                    s_x3[0:64, 512 * sbk: 512 * (sbk + 1)], ps2[:],
                    mybir.ActivationFunctionType.Relu, bias=s_b2[:, 0:1])

            def fc_sb(sbk):
                for t in range(4 * sbk, 4 * sbk + 4):
                    psf = pf.tile([128, CHORD_SIZE], F32, tag="pf")
                    nc.tensor.matmul(psf[:], s_x3[:, 128 * t: 128 * (t + 1)],
                                     s_fcwb[:, 0:CHORD_SIZE], start=True, stop=True)
                    o = ob.tile([128, CHORD_SIZE], F32, tag="o")
                    v.tensor_copy(o[:], psf[:])
                    # gpsimd queue: all scatters precede these; scatters are
                    # done (~70us) before the first fc output (~80us)
                    nc.gpsimd.dma_start(d_out.ap()[128 * t: 128 * (t + 1), :],
                                        o[:])

            return ne_build, conv1_sb, conv2_sb, fc_sb

        with tc.tile_pool(name="pn", bufs=3, space="PSUM") as ppn, \
             tc.tile_pool(name="p1", bufs=2, space="PSUM") as pp1, \
             tc.tile_pool(name="p2", bufs=1, space="PSUM") as pp2, \
             tc.tile_pool(name="pf", bufs=2, space="PSUM") as pf, \
             tc.tile_pool(name="ob", bufs=8) as ob:

            def hist_sb(sbk):
                # deferred fill-phase chord transposes: sync runs them before
                # this s-block's note transposes, keeping the hb ring (8) safe
                # for the scatters emitted right after
                if sbk in (2, 3):
                    for t in range(4 * (sbk - 2), 4 * (sbk - 2) + 4):
                        hist_tile_chord(t)
                for t in range(4 * sbk, 4 * sbk + 4):
                    hist_tile(t)

            ne_build, conv1_sb, conv2_sb, fc_sb = sblock_stages(
                lambda sbk: ct3[sbk])

            head_prep(0, 2)
            head_prep(2, 4)
            hist_sb(0)
            head_prep(4, 8)
            hist_sb(1)
            static_halo_zeros()
            # all remaining prep + hist upfront: every engine queue is in a
            # dependency-feasible order and self-paces on exact RAW/WAR deps
            for a in range(8, NT, 4):
                head_prep(a, a + 4)
                hist_sb(a // 4)
            ne_build(0)
            ne_build(1)
            # deep skew: every stage's inputs are >=1 full iteration old, so no
            # PE <-> Scalar round-trip sits on the matmul critical path
            for sbk in range(2, NSB):
                ne_build(sbk)
                conv1_sb(sbk - 2)
                if sbk >= 4:
                    conv2_sb(sbk - 4)
                if sbk >= 5:
                    fc_sb(sbk - 5)
            conv1_sb(NSB - 2)
            conv2_sb(NSB - 4)
            fc_sb(NSB - 5)
            conv1_sb(NSB - 1)
            conv2_sb(NSB - 3)
            fc_sb(NSB - 4)
            conv2_sb(NSB - 2)
            fc_sb(NSB - 3)
            conv2_sb(NSB - 1)
            fc_sb(NSB - 2)
            fc_sb(NSB - 1)

    nc.compile()
    return nc


_NC = None


def _get_nc():
    global _NC
    if _NC is None:
        _NC = _build_program()
    return _NC


def _host_prep(chord_emb, note_emb, conv1_w, conv1_b, conv2_w, conv2_b, fc_w, fc_b):
    """Shared (replicated) constant tensors."""
    note_emb = np.asarray(note_emb, np.float32)
    chord_emb = np.asarray(chord_emb, np.float32)
    w1 = np.asarray(conv1_w, np.float32)          # [256 out, 512 in, 3]

    fne = np.zeros((NBN, 256), np.float32)
    fne[0:NOTE_SIZE] = note_emb
    fne_t = np.ascontiguousarray(
        fne.reshape(NQN, 128, 2, 128).transpose(1, 0, 2, 3),
        np.float16).reshape(128, -1)

    w1n = w1[:, 256:512, :]                        # [out, in, k]
    w1ne = np.ascontiguousarray(
        w1n.transpose(1, 2, 0).reshape(2, 128, 3, 2, 128).transpose(1, 0, 2, 3, 4),
        np.float16).reshape(128, -1)

    fce = np.zeros((NBC, 256), np.float32)
    for k in range(3):
        fce[150 * k: 150 * k + CHORD_SIZE] = chord_emb @ w1[:, 0:256, k].T
    fce_t = np.ascontiguousarray(
        fce.reshape(NQC, 128, 2, 128).transpose(1, 0, 2, 3),
        np.float16).reshape(128, -1)

    w2 = np.asarray(conv2_w, np.float32).reshape(64, 2, 128, 3)
    w2t = np.ascontiguousarray(w2.transpose(2, 3, 1, 0), np.float16).reshape(128, -1)

    fcwb = np.zeros((65, 152), np.float16)
    fcwb[0:64, 0:CHORD_SIZE] = np.asarray(fc_w, np.float16)
    fcwb[64, 0:CHORD_SIZE] = np.asarray(fc_b, np.float16)

    b1t = np.ascontiguousarray(
        np.asarray(conv1_b, np.float32).reshape(2, 128).T)
    b2t = np.asarray(conv2_b, np.float32).reshape(64, 1)

    jj = np.arange(16, dtype=np.float16)
    ut = (jj[None, :] >= jj[:, None]).astype(np.float16).reshape(-1)   # j' >= j
    lt = (jj[None, :] < jj[:, None]).astype(np.float16).reshape(-1)    # j' < j
    prepc = np.zeros((128, 528), np.float16)
    prepc[:, 0:256] = ut[None, :]
    prepc[:, 256:512] = lt[None, :]
    prepc[:, 512:528] = jj[None, :]

    onesr = np.ones((1, P), np.float16)
    return fne_t, w1ne, fce_t, w2t, fcwb, b1t, b2t, prepc, onesr


def build_in_maps(chord_emb, note_emb, conv1_w, conv1_b, conv2_w, conv2_b,
                  fc_w, fc_b, note, chord):
    fne_t, w1ne, fce_t, w2t, fcwb, b1t, b2t, prepc, onesr = _host_prep(
        chord_emb, note_emb, conv1_w, conv1_b, conv2_w, conv2_b, fc_w, fc_b)
    note = np.asarray(note)
    chord = np.asarray(chord)
    in_maps = []
    for c in range(NCORES):
        nf = note[BLOC * c: BLOC * (c + 1)].reshape(P, N).astype(np.float16)
        cf = chord[BLOC * c: BLOC * (c + 1)].reshape(BLOC, S).astype(np.int64)
        note16 = np.ascontiguousarray(
            nf.reshape(NT, 128, 16).transpose(1, 0, 2)).reshape(128, -1)
        # host-shifted chord scatter bins (conv taps 0/1/2 <- s-1 / s / s+1),
        # bin = NBN + 150*k + chord value; -1 (ignored) at sequence edges
        prv = np.full((BLOC, S), -1, np.int64); prv[:, 1:] = cf[:, :-1] + NBN
        cur = cf + NBN + 150
        nxt = np.full((BLOC, S), -1, np.int64); nxt[:, :-1] = cf[:, 1:] + NBN + 300
        csidx = np.stack([prv.reshape(P), cur.reshape(P), nxt.reshape(P)],
                         axis=1)  # [P, 3]
        csidx = np.ascontiguousarray(
            csidx.reshape(NT, 128, 3).transpose(1, 0, 2).astype(np.int16)
        ).reshape(128, -1)
        in_maps.append({
            "note16": note16, "csidx": csidx, "fne": fne_t, "w1ne": w1ne,
            "fce": fce_t, "w2t": w2t, "fcwb": fcwb, "b1t": b1t, "b2t": b2t,
            "prepc": prepc, "onesr": onesr,
        })
    return in_maps


def kernel(chord_emb, note_emb, conv1_w, conv1_b, conv2_w, conv2_b, fc_w, fc_b,
           note, chord):
    nc = _get_nc()
    in_maps = build_in_maps(chord_emb, note_emb, conv1_w, conv1_b,
                            conv2_w, conv2_b, fc_w, fc_b, note, chord)
    res = run_bass_kernel_spmd(nc, in_maps, list(range(NCORES)))
    outs = [res.results[c]["out"].reshape(BLOC, S, CHORD_SIZE)
            for c in range(NCORES)]
    return np.concatenate(outs, axis=0).astype(np.float32)


# revision 14
# speedup vs baseline: 1.0083x; 1.0083x over previous
"""Trainium2 Bass kernel for nn_Chord_Note_Conv — v2 (restructured conv1).

v1 folded both embedding tables into conv1, paying a 1024-bin x 3-tap x
256-out contraction per position (48 matmuls / 512-pos s-block).  v2 cuts PE
cycles ~25% by splitting the work:
  * note path: 896-bin one-hot matmul builds ne[256] once (14 mm/s-block),
    then a dense 256-ch 3-tap conv1 (12 mm/s-block);
  * chord path: a 450-bin "concat 3 taps" one-hot (prev/cur/next chord per
    position, host-shifted pre-offset indices) folded with E_c@W1_k,
    contracted column-exact (8 mm/s-block);
  * conv2 + fc unchanged.  Per-s-block matmuls: 58 -> 44.
One GPSIMD local_scatter per 128-pos tile builds all 1408 one-hot bins
(16 deduped note slots + 3 chord slots); HWDGE xbar transpose moves them to
bin-major.  Index prep runs entirely in fp16 on DVE via a 16x16 outer-compare
(few big ops — DVE is dispatch-bound on small ones).  All inter-stage buffers
are persistent SBUF (tile pools tied producer engines to PE progress), the
pipeline is skewed so every stage's inputs are a full iteration old, and halo
guard copies ride the Scalar queue that produces them.
"""

import os

os.environ.setdefault("MYCRO_LOCAL_CACHE", "1")

import numpy as np

try:
    import concourse.bass as bass  # noqa: F401
except ImportError:
    import sys

    sys.path.insert(0, "/opt/trn_rl_repo")
    import concourse.bass as bass

from concourse import bacc, mybir, tile
from concourse.bass_utils import run_bass_kernel_spmd

FP16 = mybir.dt.float16   # NB: bf16 produced wrong results on HW (and was no faster)
F32 = mybir.dt.float32
I16 = mybir.dt.int16
ALU = mybir.AluOpType

CHORD_SIZE, NOTE_SIZE = 150, 832
B, S, N = 16, 2048, 16
NCORES = 8
BLOC = B // NCORES          # batch rows per core
P = BLOC * S                # positions per core = 4096
NT = P // 128               # pos tiles per core = 32
NSB = P // 512              # s-blocks per core = 8
SB_PER_BATCH = S // 512     # 4
NBN = 896                   # note bins (832 + pad to 7*128)
NQN = NBN // 128            # 7 note chunks
NBC = 512                   # chord-concat bins (3*150 + pad to 4*128)
NQC = NBC // 128            # 4 chord chunks
NB = NBN + NBC              # 1408 total bins = 11 chunks
NQ = NB // 128              # 11
CW = 544                    # ne/x2 s-block buffer width (16 guard + 512 + pad)


def _build_program():
    nc = bacc.Bacc("TRN2", target_bir_lowering=False, debug=False,
                   enable_asserts=False, num_devices=NCORES)

    # ---- DRAM I/O (flat 2D) ----
    d_note = nc.dram_tensor("note16", [128, NT * 16], FP16, kind="ExternalInput")
    d_csidx = nc.dram_tensor("csidx", [128, 3 * NT], I16, kind="ExternalInput")
    d_fne = nc.dram_tensor("fne", [128, NQN * 2 * 128], FP16, kind="ExternalInput")
    d_w1ne = nc.dram_tensor("w1ne", [128, 2 * 3 * 2 * 128], FP16, kind="ExternalInput")
    d_fce = nc.dram_tensor("fce", [128, NQC * 2 * 128], FP16, kind="ExternalInput")
    d_w2t = nc.dram_tensor("w2t", [128, 3 * 2 * 64], FP16, kind="ExternalInput")
    d_fcwb = nc.dram_tensor("fcwb", [65, 152], FP16, kind="ExternalInput")
    d_b1 = nc.dram_tensor("b1t", [128, 2], F32, kind="ExternalInput")
    d_b2 = nc.dram_tensor("b2t", [64, 1], F32, kind="ExternalInput")
    d_prepc = nc.dram_tensor("prepc", [128, 528], FP16, kind="ExternalInput")
    d_ones = nc.dram_tensor("onesr", [1, P], FP16, kind="ExternalInput")
    d_out = nc.dram_tensor("out", [P, CHORD_SIZE], F32, kind="ExternalOutput")

    # ---- persistent SBUF ----
    def sb(name, shape, dt):
        return nc.alloc_sbuf_tensor(name, list(shape), dt).ap()

    s_note = sb("s_note", [128, NT * 16], FP16)
    s_csidx = sb("s_csidx", [128, 3 * NT], I16)
    s_fne = sb("s_fne", [128, NQN * 2 * 128], FP16)
    s_w1ne = sb("s_w1ne", [128, 2 * 3 * 2 * 128], FP16)
    s_fce = sb("s_fce", [128, NQC * 2 * 128], FP16)
    s_w2t = sb("s_w2t", [128, 3 * 2 * 64], FP16)
    s_fcwb = sb("s_fcwb", [65, 152], FP16)
    s_b1 = sb("s_b1", [128, 2], F32)
    s_b2 = sb("s_b2", [64, 1], F32)
    s_prepc = sb("s_prepc", [128, 528], FP16)
    s_eqall = sb("s_eqall", [128, 4 * 256], FP16)
    s_eqall2 = sb("s_eqall2", [128, 4 * 256], FP16)
    s_x3 = sb("s_x3", [65, P], FP16)
    s_mask = sb("s_mask", [128, NT * 16], FP16)
    s_val = sb("s_val", [128, NT * 16], FP16)
    s_isdup = sb("s_isdup", [128, NT * 16], FP16)
    s_eq = sb("s_eq", [128, NT * 16], FP16)
    s_tmp = sb("s_tmp", [128, NT * 16], FP16)
    s_cnt = sb("s_cnt", [128, NT], F32)
    s_inv = sb("s_inv", [128, NT], F32)
    s_inv16 = sb("s_inv16", [128, NT], FP16)
    s_sidx = sb("s_sidx", [128, NT * 20], I16)
    s_sval = sb("s_sval", [128, NT * 20], FP16)
    # per-s-block persistent: ne [128, 2q, CW], x2 [128, 2q, CW]
    s_ne = [sb(f"s_ne{i}", [128, 2 * CW], FP16) for i in range(NSB)]
    s_x2 = [sb(f"s_x2{i}", [128, 2 * CW], FP16) for i in range(NSB)]
    # persistent hist-path buffers (pools here would tie scatter/transpose
    # recycling to PE progress via conservative pool-WAR sync)
    s_hb = [sb(f"s_hb{i}", [128, NB], FP16) for i in range(8)]
    s_ct = [sb(f"s_ct{i}", [128, NQ * 512], FP16) for i in range(NSB)]

    note3 = s_note.rearrange("p (c j) -> p c j", j=16)
    mask3 = s_mask.rearrange("p (c j) -> p c j", j=16)
    val3 = s_val.rearrange("p (c j) -> p c j", j=16)
    isd3 = s_isdup.rearrange("p (c j) -> p c j", j=16)
    eq3 = s_eq.rearrange("p (c j) -> p c j", j=16)
    tmp3 = s_tmp.rearrange("p (c j) -> p c j", j=16)
    sidx3 = s_sidx.rearrange("p (c j) -> p c j", j=20)
    sval3 = s_sval.rearrange("p (c j) -> p c j", j=20)
    csidx3 = s_csidx.rearrange("p (c k) -> p c k", k=3)
    fne4 = s_fne.rearrange("p (q o m) -> p q o m", q=NQN, o=2)
    w1ne5 = s_w1ne.rearrange("p (q k o m) -> p q k o m", q=2, k=3, o=2)
    fce4 = s_fce.rearrange("p (q o m) -> p q o m", q=NQC, o=2)
    w2t4 = s_w2t.rearrange("p (k q o) -> p k q o", k=3, q=2)
    ne3 = [t.rearrange("p (q w) -> p q w", w=CW) for t in s_ne]
    x23 = [t.rearrange("p (q w) -> p q w", w=CW) for t in s_x2]
    ct3 = [t.rearrange("p (q w) -> p q w", w=512) for t in s_ct]

    with tile.TileContext(nc) as tc, \
         nc.allow_low_precision(reason="int16 counts <=16 are exact; fp16 data"):
        v = nc.vector
        # ---- input loads (latency-critical first) ----
        nc.sync.dma_start(s_note, d_note.ap())
        nc.sync.dma_start(s_csidx, d_csidx.ap())
        nc.sync.dma_start(s_prepc, d_prepc.ap())
        nc.sync.dma_start(s_fne, d_fne.ap())
        nc.sync.dma_start(s_fce, d_fce.ap())
        nc.sync.dma_start(s_w1ne, d_w1ne.ap())
        nc.sync.dma_start(s_w2t, d_w2t.ap())
        nc.sync.dma_start(s_fcwb, d_fcwb.ap())
        nc.sync.dma_start(s_b1, d_b1.ap())
        nc.sync.dma_start(s_b2, d_b2.ap())
        nc.sync.dma_start(s_x3[64:65, :], d_ones.ap())  # fc bias ones row

        # chord scatter slots: idx from host (pre-offset, -1 at boundaries),
        # val 1.0; slot 19 is inert padding
        v.tensor_copy(sidx3[:, :, 16:19], csidx3)
        v.memset(sval3[:, :, 16:19], 1.0)
        v.memset(sidx3[:, :, 19:20], -1)
        v.memset(sval3[:, :, 19:20], 0.0)

        def static_halo_zeros():
            # batch-boundary halo zeros (never rewritten)
            for k in range(NSB):
                if k % SB_PER_BATCH == 0:
                    v.memset(ne3[k][:, :, 15:16], 0.0)
                    v.memset(x23[k][:, :, 15:16], 0.0)
                if k % SB_PER_BATCH == SB_PER_BATCH - 1:
                    v.memset(ne3[k][:, :, 528:529], 0.0)
                    v.memset(x23[k][:, :, 528:529], 0.0)

        # ---- index prep ----
        def finish_prep(a, b):
            """common tail: fv, scat_idx, scat_val from mask/val/isdup."""
            w_ = b - a
            nt = note3[:, a:b, :]
            mk = mask3[:, a:b, :]
            isd = isd3[:, a:b, :]
            eq = eq3[:, a:b, :]
            tp = tmp3[:, a:b, :]
            v.tensor_tensor(tp, mk, isd, ALU.mult)
            v.tensor_tensor(isd, mk, tp, ALU.subtract)   # fv
            v.tensor_tensor(tp, nt, isd, ALU.mult)
            v.tensor_scalar_add(eq, isd, -1)
            v.tensor_tensor(sidx3[:, a:b, 0:16], tp, eq, ALU.add)
            inv_b = s_inv16[:, a:b].unsqueeze(2).broadcast_to((128, w_, 16))
            return v.tensor_tensor(sval3[:, a:b, 0:16], val3[:, a:b, :], inv_b,
                                   ALU.mult)

        ut2 = s_prepc[:, 0:256].rearrange("p (a b) -> p a b", b=16)    # 1[j' >= j]
        lt2 = s_prepc[:, 256:512].rearrange("p (a b) -> p a b", b=16)  # 1[j' < j]
        jidx = s_prepc[:, 512:528]                                     # iota 0..15

        def head_prep(a, b):
            """Low-latency variant: 16x16 outer compare in few big ops."""
            w_ = b - a
            nt = note3[:, a:b, :]
            mk = mask3[:, a:b, :]
            eq = eq3[:, a:b, :]
            tp = tmp3[:, a:b, :]
            eqa = s_eqall.rearrange("p (c i j) -> p c i j", i=16, j=16)[:, 0:w_, :, :]
            eqb = s_eqall2.rearrange("p (c i j) -> p c i j", i=16, j=16)[:, 0:w_, :, :]
            jid_b = jidx.unsqueeze(1).broadcast_to((128, w_, 16))
            # first-zero index -> mask, cnt
            v.tensor_scalar(eq, nt, 0, None, ALU.not_equal)            # nz
            v.scalar_tensor_tensor(tp, eq, 16, jid_b, ALU.mult, ALU.add)
            v.tensor_reduce(s_cnt[:, a:b], tp, mybir.AxisListType.X, ALU.min)
            v.tensor_tensor(mk, jid_b,
                            s_cnt[:, a:b].unsqueeze(2).broadcast_to((128, w_, 16)),
                            ALU.is_lt)
            v.tensor_scalar(s_cnt[:, a:b], s_cnt[:, a:b], 16.0, 1.0, ALU.min, ALU.max)
            v.reciprocal(s_inv[:, a:b], s_cnt[:, a:b])
            v.tensor_copy(s_inv16[:, a:b], s_inv[:, a:b])
            # 16x16 equality outer product
            v.tensor_tensor(eqa,
                            nt.unsqueeze(3).broadcast_to((128, w_, 16, 16)),
                            nt.unsqueeze(2).broadcast_to((128, w_, 16, 16)),
                            ALU.is_equal)
            # t = m_j' * eq ; val_j = sum_{j'>=j} t ; isdup_j = max_{j'<j} t
            v.tensor_tensor(eqa, eqa,
                            mask3[:, a:b, :].unsqueeze(2).broadcast_to((128, w_, 16, 16)),
                            ALU.mult)
            v.tensor_tensor(eqb, eqa,
                            lt2.unsqueeze(1).broadcast_to((128, w_, 16, 16)), ALU.mult)
            v.tensor_reduce(isd3[:, a:b, :], eqb, mybir.AxisListType.X, ALU.max)
            v.tensor_tensor(eqa, eqa,
                            ut2.unsqueeze(1).broadcast_to((128, w_, 16, 16)), ALU.mult)
            v.tensor_reduce(val3[:, a:b, :], eqa, mybir.AxisListType.X, ALU.add)
            return finish_prep(a, b)

        # ---- histogram build (scatter notes+chord) + transpose
        # NB: all transposes must stay on the sync queue — issuing some
        # from the Scalar engine produced silent data races twice.
        # The chord-bin transposes are deferred ~2 s-blocks: the PE needs
        # note chunks (ne_build) ~20us before chord chunks (conv1), so
        # note-only transposes (896 cols, ~1.0us) feed the fill phase faster
        # than full-width ones (1408 cols, ~1.55us).
        def hist_tile(t):
            """One 128-pos tile: build [128, 1408] one-hot rows, transpose into
            the s-block ct buffer at cols 128*(t%4).  First two s-blocks:
            note bins only (chord deferred) — shorter ops feed ne_build(0/1)
            sooner; the ~700ns fixed cost per extra transpose op makes the
            split a loss in steady state, so later tiles go full-width."""
            hb = s_hb[t % 8]
            nc.gpsimd.local_scatter(
                hb, sval3[:, t, :], sidx3[:, t, :],
                channels=128, num_elems=NB, num_idxs=20)
            tq = t % 4
            if t < 8:
                nc.sync.dma_start_transpose(
                    ct3[t // 4][:, 0:NQN, 128 * tq: 128 * (tq + 1)],
                    hb[:, 0:NBN])
            else:
                nc.sync.dma_start_transpose(
                    ct3[t // 4][:, :, 128 * tq: 128 * (tq + 1)], hb)

        def hist_tile_chord(t):
            tq = t % 4
            nc.sync.dma_start_transpose(
                ct3[t // 4][:, NQN:NQ, 128 * tq: 128 * (tq + 1)],
                s_hb[t % 8][:, NBN:NB])

        # ---- conv/fc stages ----
        def sblock_stages(ct_of):
            def ne_build(sbk):
                ct = ct_of(sbk)
                # sblock 0: column-halves so the very first matmul group only
                # needs transposes t0/t1 (starts the PE ~4us earlier; extending
                # the split to sblock 1 measured slightly worse)
                cols = ((0, 256), (256, 512)) if sbk == 0 else ((0, 512),)
                for oc in range(2):
                    for a, b in cols:
                        ps = ppn.tile([128, b - a], F32, tag="pn")
                        for q in range(NQN):
                            nc.tensor.matmul(ps[:], fne4[:, q, oc, :],
                                             ct[:, q, a:b],
                                             start=(q == 0),
                                             stop=(q == NQN - 1))
                        nc.scalar.copy(ne3[sbk][:, oc, 16 + a: 16 + b], ps[:])
                # halo guards on Scalar (all deps already satisfied here)
                if sbk % SB_PER_BATCH != 0:
                    nc.scalar.copy(ne3[sbk][:, :, 15:16],
                                   ne3[sbk - 1][:, :, 527:528])
                    nc.scalar.copy(ne3[sbk - 1][:, :, 528:529],
                                   ne3[sbk][:, :, 16:17])

            def conv1_sb(sbk):
                ct = ct_of(sbk)
                for co in range(2):
                    ps = pp1.tile([128, 512], F32, tag="p1")
                    mms = [("c", q, 0) for q in range(NQC)] + \
                          [("n", q, k) for q in range(2) for k in range(3)]
                    for i, (kind, q, k) in enumerate(mms):
                        if kind == "c":
                            lhs = fce4[:, q, co, :]
                            rhs = ct[:, NQN + q, 0:512]
                        else:
                            lhs = w1ne5[:, q, k, co, :]
                            rhs = ne3[sbk][:, q, 15 + k: 527 + k]
                        nc.tensor.matmul(ps[:], lhs, rhs,
                                         start=(i == 0), stop=(i == len(mms) - 1))
                    nc.scalar.activation(
                        x23[sbk][:, co, 16:528], ps[:],
                        mybir.ActivationFunctionType.Relu,
                        bias=s_b1[:, co:co + 1])
                if sbk % SB_PER_BATCH != 0:
                    nc.scalar.copy(x23[sbk][:, :, 15:16],
                                   x23[sbk - 1][:, :, 527:528])
                    nc.scalar.copy(x23[sbk - 1][:, :, 528:529],
                                   x23[sbk][:, :, 16:17])

            def conv2_sb(sbk):
                ps2 = pp2.tile([64, 512], F32, tag="p2")
                mms = [(1, 0), (0, 0), (2, 0), (0, 1), (1, 1), (2, 1)]
                for i, (k, q) in enumerate(mms):
                    nc.tensor.matmul(
                        ps2[:], w2t4[:, k, q, :],
                        x23[sbk][:, q, 15 + k: 527 + k],
                        start=(i == 0), stop=(i == len(mms) - 1))
                nc.scalar.activation(
                    s_x3[0:64, 512 * sbk: 512 * (sbk + 1)], ps2[:],
                    mybir.ActivationFunctionType.Relu, bias=s_b2[:, 0:1])

            def fc_sb(sbk):
                for t in range(4 * sbk, 4 * sbk + 4):
                    psf = pf.tile([128, CHORD_SIZE], F32, tag="pf")
                    nc.tensor.matmul(psf[:], s_x3[:, 128 * t: 128 * (t + 1)],
                                     s_fcwb[:, 0:CHORD_SIZE], start=True, stop=True)
                    o = ob.tile([128, CHORD_SIZE], F32, tag="o")
                    v.tensor_copy(o[:], psf[:])
                    # gpsimd queue: all scatters precede these; scatters are
                    # done (~70us) before the first fc output (~80us)
                    nc.gpsimd.dma_start(d_out.ap()[128 * t: 128 * (t + 1), :],
                                        o[:])

            return ne_build, conv1_sb, conv2_sb, fc_sb

        with tc.tile_pool(name="pn", bufs=3, space="PSUM") as ppn, \
             tc.tile_pool(name="p1", bufs=2, space="PSUM") as pp1, \
             tc.tile_pool(name="p2", bufs=1, space="PSUM") as pp2, \
             tc.tile_pool(name="pf", bufs=2, space="PSUM") as pf, \
             tc.tile_pool(name="ob", bufs=8) as ob:

            def hist_sb(sbk):
                # deferred fill-phase chord transposes: sync runs them before
                # this s-block's note transposes, keeping the hb ring (8) safe
                # for the scatters emitted right after
                if sbk in (2, 3):
                    for t in range(4 * (sbk - 2), 4 * (sbk - 2) + 4):
                        hist_tile_chord(t)
                for t in range(4 * sbk, 4 * sbk + 4):
                    hist_tile(t)

            ne_build, conv1_sb, conv2_sb, fc_sb = sblock_stages(
                lambda sbk: ct3[sbk])

            head_prep(0, 2)
            head_prep(2, 4)
            hist_sb(0)
            head_prep(4, 8)
            hist_sb(1)
            static_halo_zeros()
            # all remaining prep + hist upfront: every engine queue is in a
            # dependency-feasible order and self-paces on exact RAW/WAR deps
            for a in range(8, NT, 4):
                head_prep(a, a + 4)
                hist_sb(a // 4)
            ne_build(0)
            ne_build(1)
            # deep skew: every stage's inputs are >=1 full iteration old, so no
            # PE <-> Scalar round-trip sits on the matmul critical path
            for sbk in range(2, NSB):
                ne_build(sbk)
                conv1_sb(sbk - 2)
                if sbk >= 4:
                    conv2_sb(sbk - 4)
                if sbk >= 5:
                    fc_sb(sbk - 5)
            conv1_sb(NSB - 2)
            conv2_sb(NSB - 4)
            fc_sb(NSB - 5)
            conv1_sb(NSB - 1)
            conv2_sb(NSB - 3)
            fc_sb(NSB - 4)
            conv2_sb(NSB - 2)
            fc_sb(NSB - 3)
            conv2_sb(NSB - 1)
            fc_sb(NSB - 2)
            fc_sb(NSB - 1)

    nc.compile()
    return nc


_NC = None


def _get_nc():
    global _NC
    if _NC is None:
        _NC = _build_program()
    return _NC


def _host_prep(chord_emb, note_emb, conv1_w, conv1_b, conv2_w, conv2_b, fc_w, fc_b):
    """Shared (replicated) constant tensors."""
    note_emb = np.asarray(note_emb, np.float32)
    chord_emb = np.asarray(chord_emb, np.float32)
    w1 = np.asarray(conv1_w, np.float32)          # [256 out, 512 in, 3]

    fne = np.zeros((NBN, 256), np.float32)
    fne[0:NOTE_SIZE] = note_emb
    fne_t = np.ascontiguousarray(
        fne.reshape(NQN, 128, 2, 128).transpose(1, 0, 2, 3),
        np.float16).reshape(128, -1)

    w1n = w1[:, 256:512, :]                        # [out, in, k]
    w1ne = np.ascontiguousarray(
        w1n.transpose(1, 2, 0).reshape(2, 128, 3, 2, 128).transpose(1, 0, 2, 3, 4),
        np.float16).reshape(128, -1)

    fce = np.zeros((NBC, 256), np.float32)
    for k in range(3):
        fce[150 * k: 150 * k + CHORD_SIZE] = chord_emb @ w1[:, 0:256, k].T
    fce_t = np.ascontiguousarray(
        fce.reshape(NQC, 128, 2, 128).transpose(1, 0, 2, 3),
        np.float16).reshape(128, -1)

    w2 = np.asarray(conv2_w, np.float32).reshape(64, 2, 128, 3)
    w2t = np.ascontiguousarray(w2.transpose(2, 3, 1, 0), np.float16).reshape(128, -1)

    fcwb = np.zeros((65, 152), np.float16)
    fcwb[0:64, 0:CHORD_SIZE] = np.asarray(fc_w, np.float16)
    fcwb[64, 0:CHORD_SIZE] = np.asarray(fc_b, np.float16)

    b1t = np.ascontiguousarray(
        np.asarray(conv1_b, np.float32).reshape(2, 128).T)
    b2t = np.asarray(conv2_b, np.float32).reshape(64, 1)

    jj = np.arange(16, dtype=np.float16)
    ut = (jj[None, :] >= jj[:, None]).astype(np.float16).reshape(-1)   # j' >= j
    lt = (jj[None, :] < jj[:, None]).astype(np.float16).reshape(-1)    # j' < j
    prepc = np.zeros((128, 528), np.float16)
    prepc[:, 0:256] = ut[None, :]
    prepc[:, 256:512] = lt[None, :]
    prepc[:, 512:528] = jj[None, :]

    onesr = np.ones((1, P), np.float16)
    return fne_t, w1ne, fce_t, w2t, fcwb, b1t, b2t, prepc, onesr


def build_in_maps(chord_emb, note_emb, conv1_w, conv1_b, conv2_w, conv2_b,
                  fc_w, fc_b, note, chord):
    fne_t, w1ne, fce_t, w2t, fcwb, b1t, b2t, prepc, onesr = _host_prep(
        chord_emb, note_emb, conv1_w, conv1_b, conv2_w, conv2_b, fc_w, fc_b)
    note = np.asarray(note)
    chord = np.asarray(chord)
    in_maps = []
    for c in range(NCORES):
        nf = note[BLOC * c: BLOC * (c + 1)].reshape(P, N).astype(np.float16)
        cf = chord[BLOC * c: BLOC * (c + 1)].reshape(BLOC, S).astype(np.int64)
        note16 = np.ascontiguousarray(
            nf.reshape(NT, 128, 16).transpose(1, 0, 2)).reshape(128, -1)
        # host-shifted chord scatter bins (conv taps 0/1/2 <- s-1 / s / s+1),
        # bin = NBN + 150*k + chord value; -1 (ignored) at sequence edges
        prv = np.full((BLOC, S), -1, np.int64); prv[:, 1:] = cf[:, :-1] + NBN
        cur = cf + NBN + 150
        nxt = np.full((BLOC, S), -1, np.int64); nxt[:, :-1] = cf[:, 1:] + NBN + 300
        csidx = np.stack([prv.reshape(P), cur.reshape(P), nxt.reshape(P)],
                         axis=1)  # [P, 3]
        csidx = np.ascontiguousarray(
            csidx.reshape(NT, 128, 3).transpose(1, 0, 2).astype(np.int16)
        ).reshape(128, -1)
        in_maps.append({
            "note16": note16, "csidx": csidx, "fne": fne_t, "w1ne": w1ne,
            "fce": fce_t, "w2t": w2t, "fcwb": fcwb, "b1t": b1t, "b2t": b2t,
            "prepc": prepc, "onesr": onesr,
        })
    return in_maps


def kernel(chord_emb, note_emb, conv1_w, conv1_b, conv2_w, conv2_b, fc_w, fc_b,
           note, chord):
    nc = _get_nc()
    in_maps = build_in_maps(chord_emb, note_emb, conv1_w, conv1_b,
                            conv2_w, conv2_b, fc_w, fc_b, note, chord)
    res = run_bass_kernel_spmd(nc, in_maps, list(range(NCORES)))
    outs = [res.results[c]["out"].reshape(BLOC, S, CHORD_SIZE)
            for c in range(NCORES)]
    return np.concatenate(outs, axis=0).astype(np.float32)
